# revision 30
# baseline (speedup 1.0000x reference)
"""Multi-head causal attention kernel for Trainium2 (8 NeuronCores).

Problem: B=4, S=2048, HID=1024, H=16 heads (head_dim 64), causal mask,
fp32 I/O.  out = softmax(mask + (XqWq)(XkWk)^T/8) (XvWv) Wo

Sharding: 8 cores = 4 batches x 2 head-groups.  Core c handles batch
c//2 and heads (c%2)*8 .. +8 (dk slice of 512).  Each core computes a
full-shape [S, HID] partial output (its head-group's contribution
through Wo); the host sums the two partials per batch.

Host-side prep: X tensors are transposed to [HID, S] and cast to bf16
(with Wq pre-scaled by 1/8) so the kernel needs no PE transposes and
half the HBM traffic.  All matmul operands are bf16 (1 cycle/column on
the PE vs ~2 for fp32); accumulation stays fp32 in PSUM, mask add and
softmax normalization stay fp32, output is fp32.

Per-core dataflow:
  - Project in s-tiles of 512: qT (per-window, [e,s]), kT (persistent
    [e,s], 2 heads per 128-partition tile), v (persistent [s,e] with a
    ones column per head so the PV matmul also emits softmax
    denominators).
  - Attention in transposed [k,q] orientation per (q-window j, head
    pair hp): logits^T = kT-chunk (stationary) x qT (moving) with
    causally-restricted columns; additive triangular mask on diagonal
    blocks (DVE, in PSUM); exp on ScalarE PSUM->SBUF (bf16 out); PV
    accumulates ctx^T in PSUM.  Denominator rows -> one batched
    reciprocal (DVE) -> partition_broadcast (GpSimd) ->
    multiply-evacuate ctx^T to bf16 (DVE).
  - Output projection ctx^T.T @ Wo per q-window; PSUM evacuation on
    DVE (ScalarE stays dedicated to exp); bf16 DMA out, fp32 host sum.

Emission order = Tile scheduling priority.  Projection pieces for
s-tile j+1 interleave with the attention units of window j, so the PE
always has dense independent matmul work while the exp-bound attention
chain waits on ScalarE (keeping the PE HAM activity monitor at the
full 2.4 GHz clock).  All out-projections are deferred to window 3 —
the only window with no projection work left — and the final window
rotates its head-pair order so the last unit's softmax-normalization
chain hides under out-block matmuls whose contraction order puts the
late head-pair last.
"""

import numpy as np

B, S, HID = 4, 2048, 1024
H_LOCAL, E_LOCAL = 8, 512  # heads / dk columns handled per core
N_CORES = 8

_cached = {}


def _build():
    from concourse import bacc
    import concourse.bass as bass
    import concourse.mybir as mybir
    import concourse.tile as tile

    F32 = mybir.dt.float32
    BF16 = mybir.dt.bfloat16
    Exp = mybir.ActivationFunctionType.Exp

    nc = bacc.Bacc()
    # pre-transposed [HID, S] bf16 inputs
    xq = nc.dram_tensor("xq", [HID, S], BF16, kind="ExternalInput")
    xk = nc.dram_tensor("xk", [HID, S], BF16, kind="ExternalInput")
    xv = nc.dram_tensor("xv", [HID, S], BF16, kind="ExternalInput")
    wq = nc.dram_tensor("wq", [HID, E_LOCAL], BF16, kind="ExternalInput")
    wk = nc.dram_tensor("wk", [HID, E_LOCAL], BF16, kind="ExternalInput")
    wv = nc.dram_tensor("wv", [HID, E_LOCAL], BF16, kind="ExternalInput")
    wo = nc.dram_tensor("wo", [E_LOCAL, HID], BF16, kind="ExternalInput")
    # bf16 output: halves the store traffic; the host sums the two
    # per-batch partials in fp32 (quantization ~0.2% of partial
    # magnitude, far under the error budget)
    out = nc.dram_tensor("out", [S, HID], BF16, kind="ExternalOutput")

    NDC = HID // 128       # 8 d-chunks (contraction)
    NEC = E_LOCAL // 128   # 4 e-chunks = head pairs
    NKC = S // 128         # 16 k-chunks
    NQT = 4                # q windows of 512 = s-tiles
    STW = S // NQT         # 512

    with tile.TileContext(nc) as tc:
        with (
            tc.sbuf_pool(name="consts", bufs=1) as consts,
            tc.sbuf_pool(name="persist", bufs=1) as persist,
            tc.sbuf_pool(name="stream", bufs=1) as sm,
            tc.psum_pool(name="ps", bufs=1) as ps,
        ):
            # additive causal mask for diagonal [k,q] blocks: 0 where
            # k <= q else -1e9
            trimask = consts.tile([128, 128], F32)
            nc.gpsimd.memset(trimask, 0.0)
            nc.gpsimd.affine_select(
                out=trimask, in_=trimask,
                compare_op=mybir.AluOpType.is_ge, fill=-1e9, base=0,
                pattern=[[1, 128]], channel_multiplier=-1,
            )
            ones_col = consts.tile([128, 1], BF16)
            nc.vector.memset(ones_col, 1.0)
            # warm the ACT exp table during the initial DMA wait
            warmup = consts.tile([1, 16], F32)
            nc.vector.memset(warmup, 0.0)
            nc.scalar.activation(warmup, warmup, Exp)

            kt_sb = [persist.tile([128, S], BF16, name=f"kt{i}",
                                  tag=f"kt{i}") for i in range(NEC)]
            v_sb = [persist.tile([128, H_LOCAL, 65], BF16, name=f"v{i}",
                                 tag=f"v{i}") for i in range(NKC)]

            wq_sb = sm.tile([128, NDC, E_LOCAL], BF16, tag="wq", bufs=1)
            wk_sb = sm.tile([128, NDC, E_LOCAL], BF16, tag="wk", bufs=1)
            wv_sb = sm.tile([128, NDC, E_LOCAL], BF16, tag="wv", bufs=1)
            wo_sb = sm.tile([128, NEC, HID], BF16, tag="wo", bufs=1)

            qt_rot = {}   # (window, ec) -> [128, 512] bf16 tile
            ctx_rot = {}  # (window, hp) -> [128, 512] bf16 tile
            xt_tiles = {}  # (tensor, st) -> [128, NDC, 512] tile

            def load_w(which):
                src = {"q": (wq, wq_sb), "k": (wk, wk_sb),
                       "v": (wv, wv_sb)}.get(which)
                if src is not None:
                    nc.sync.dma_start(
                        out=src[1],
                        in_=src[0].rearrange("(dc p) e -> p dc e", p=128))
                else:
                    nc.sync.dma_start(
                        out=wo_sb,
                        in_=wo.rearrange("(dv p) n -> p dv n", p=128))

            def load_xt(tname, st):
                xdram = {"q": xq, "k": xk, "v": xv}[tname]
                t = sm.tile([128, NDC, STW], BF16, tag=f"xt{tname}",
                            bufs=2, name=f"xt_{tname}{st}")
                nc.sync.dma_start(
                    out=t,
                    in_=xdram[:, st * STW:(st + 1) * STW].rearrange(
                        "(dc p) s -> p dc s", p=128))
                xt_tiles[(tname, st)] = t

            def proj_q(st, ec):
                xt = xt_tiles[("q", st)]
                pj = ps.tile([128, STW], F32, tag="work", bufs=2,
                             name=f"pjq_{st}_{ec}")
                for dc in range(NDC):
                    nc.tensor.matmul(
                        pj, wq_sb[:, dc, ec * 128:(ec + 1) * 128],
                        xt[:, dc, :],
                        start=(dc == 0), stop=(dc == NDC - 1))
                qt_rot[(st, ec)] = sm.tile([128, STW], BF16,
                                           tag=f"qtr{ec}", bufs=4,
                                           name=f"qtr{ec}_{st}")
                nc.vector.tensor_copy(qt_rot[(st, ec)], pj)

            def proj_k(st, ec):
                xt = xt_tiles[("k", st)]
                pj = ps.tile([128, STW], F32, tag="work", bufs=2,
                             name=f"pjk_{st}_{ec}")
                for dc in range(NDC):
                    nc.tensor.matmul(
                        pj, wk_sb[:, dc, ec * 128:(ec + 1) * 128],
                        xt[:, dc, :],
                        start=(dc == 0), stop=(dc == NDC - 1))
                nc.vector.tensor_copy(
                    kt_sb[ec][:, st * STW:(st + 1) * STW], pj)

            def proj_v(st, sc):
                xt = xt_tiles[("v", st)]
                pv = ps.tile([128, E_LOCAL], F32, tag="work", bufs=2,
                             name=f"pv_{st}_{sc}")
                for dc in range(NDC):
                    nc.tensor.matmul(
                        pv, xt[:, dc, sc * 128:(sc + 1) * 128],
                        wv_sb[:, dc, :],
                        start=(dc == 0), stop=(dc == NDC - 1))
                ci = st * 4 + sc
                nc.vector.tensor_copy(
                    v_sb[ci][:, :, 0:64],
                    pv.rearrange("p (h e) -> p h e", h=H_LOCAL))
                ones_b = bass.AP(
                    tensor=ones_col.tensor, offset=ones_col.offset,
                    ap=[ones_col.ap[0], [0, H_LOCAL], ones_col.ap[1]],
                )
                nc.vector.tensor_copy(v_sb[ci][:, :, 64:65], ones_b)

            def attention_unit(j, hp):
                q0 = j * 512
                qt = qt_rot[(j, hp)]
                cpx = [ps.tile([65, 512], F32, tag="cpx", bufs=2,
                               name=f"cpx{hp}_{j}_{hi}")
                       for hi in range(2)]
                ctx_rot[(j, hp)] = sm.tile([128, 512], BF16,
                                           tag=f"ctxr{hp}", bufs=4,
                                           name=f"ctxr{hp}_{j}")
                # chunk order: c=0 first (its full-width PV write
                # clears the whole PSUM range for the accumulation
                # group), then the latency-bound diagonal chunks (they
                # overlap the previous unit's normalization chain),
                # then the remaining dense full-width chunks so the
                # unit ends with back-to-back PE work
                chunks = ([0] + [c for c in range(4 * j, 4 * j + 4)
                                 if c != 0] + list(range(1, 4 * j)))
                for ci, c in enumerate(chunks):
                    vo = max(0, c * 128 - q0)
                    lg = ps.tile([128, 1024], F32, tag="lg", bufs=2,
                                 name=f"lg{hp}_{j}_{c}")
                    pt = sm.tile([128, 1024], BF16, tag="pt", bufs=3,
                                 name=f"pt{hp}_{j}_{c}")
                    for hi in range(2):
                        nc.tensor.matmul(
                            lg[:, hi * 512 + vo:(hi + 1) * 512],
                            kt_sb[hp][hi * 64:(hi + 1) * 64,
                                      c * 128:(c + 1) * 128],
                            qt[hi * 64:(hi + 1) * 64, vo:512],
                            start=True, stop=True)
                    if c >= 4 * j:
                        m = c - 4 * j
                        blk = lg.rearrange("p (hh q) -> p hh q", hh=2)[
                            :, :, m * 128:(m + 1) * 128]
                        tri_b = bass.AP(
                            tensor=trimask.tensor, offset=trimask.offset,
                            ap=[trimask.ap[0], [0, 2], trimask.ap[1]],
                        )
                        nc.vector.tensor_add(blk, blk, tri_b)
                    if vo == 0:
                        nc.scalar.activation(pt, lg, Exp)
                    else:
                        for hi in range(2):
                            nc.scalar.activation(
                                pt[:, hi * 512 + vo:(hi + 1) * 512],
                                lg[:, hi * 512 + vo:(hi + 1) * 512], Exp)
                    for hi in range(2):
                        nc.tensor.matmul(
                            cpx[hi][:, vo:512],
                            v_sb[c][:, hp * 2 + hi, :],
                            pt[:, hi * 512 + vo:(hi + 1) * 512],
                            start=(ci == 0), stop=(ci == len(chunks) - 1))
                den = sm.tile([1, 1024], F32, tag="den", bufs=2,
                              name=f"den{hp}_{j}")
                for hi in range(2):
                    nc.vector.tensor_copy(
                        den[0:1, hi * 512:(hi + 1) * 512],
                        cpx[hi][64:65, :])
                nc.vector.reciprocal_approx_fast(out=den, in_=den)
                for hi in range(2):
                    bcast = sm.tile([64, 512], F32, tag=f"bcast{hi}",
                                    bufs=2, name=f"bc{hp}_{j}_{hi}")
                    nc.gpsimd.partition_broadcast(
                        bcast, den[0:1, hi * 512:(hi + 1) * 512])
                    nc.vector.tensor_mul(
                        ctx_rot[(j, hp)][hi * 64:(hi + 1) * 64, :],
                        cpx[hi][0:64, :], bcast)

            def out_block(qc, on_scalar=False, dvc_order=(0, 1, 2, 3)):
                for nh in range(2):
                    po = ps.tile([128, 512], F32, tag="work", bufs=2,
                                 name=f"po{qc}_{nh}")
                    for i, dvc in enumerate(dvc_order):
                        nc.tensor.matmul(
                            po,
                            ctx_rot[(qc // 4, dvc)][:,
                                                    (qc % 4) * 128:
                                                    (qc % 4 + 1) * 128],
                            wo_sb[:, dvc, nh * 512:(nh + 1) * 512],
                            start=(i == 0), stop=(i == NEC - 1))
                    osb = sm.tile([128, 512], BF16, tag="osb", bufs=4,
                                  name=f"osb{qc}_{nh}")
                    if on_scalar:
                        # tail blocks: exp is done, ScalarE is idle
                        nc.scalar.copy(osb, po)
                    else:
                        nc.vector.tensor_copy(osb, po)
                    nc.sync.dma_start(
                        out=out[qc * 128:(qc + 1) * 128,
                                nh * 512:(nh + 1) * 512],
                        in_=osb)

            # ---- emission (= scheduling priority) order ----
            # Prologue: weights + s-tile 0, with attention(0,0)'s
            # dependencies (q0/k0 head-pair 0, all v) first.  Weight
            # loads interleave with x-tile loads so the first
            # projection can start as early as possible.
            # wq + x_q tile 0 load in halves so the first projection's
            # dc 0..3 matmuls start after ~1MB of DMA instead of ~2MB
            xtq0 = sm.tile([128, NDC, STW], BF16, tag="xtq", bufs=2,
                           name="xt_q0")
            xt_tiles[("q", 0)] = xtq0
            for h in range(2):
                dcs = slice(h * 4, h * 4 + 4)
                rows = slice(h * 512, h * 512 + 512)
                nc.sync.dma_start(
                    out=wq_sb[:, dcs, :],
                    in_=wq[rows, :].rearrange("(dc p) e -> p dc e", p=128))
                nc.sync.dma_start(
                    out=xtq0[:, dcs, :],
                    in_=xq[rows, 0:STW].rearrange(
                        "(dc p) s -> p dc s", p=128))
            load_w("k")
            load_xt("k", 0)
            load_w("v")
            load_xt("v", 0)
            load_w("o")
            # all q pieces, then k, then v, matching DMA arrival order
            # (xq lands first): the PE stream is in-order, so work that
            # only needs xq must come before anything needing xk/xv
            for ec in range(NEC):
                proj_q(0, ec)
            for ec in range(NEC):
                proj_k(0, ec)
            for sc in range(4):
                proj_v(0, sc)

            for j in range(NQT):
                if j < NQT - 1:
                    st = j + 1
                    loads = [("q", st), ("k", st), ("v", st)]
                    fill = [
                        (proj_q, st, 0), (proj_k, st, 0),
                        (proj_v, st, 0), (proj_v, st, 1),
                        (proj_v, st, 2), (proj_v, st, 3),
                        (proj_q, st, 1), (proj_k, st, 1),
                        (proj_q, st, 2), (proj_k, st, 2),
                        (proj_q, st, 3), (proj_k, st, 3),
                    ]
                    hp_order = range(NEC)
                else:
                    loads = []
                    fill = []
                    # rotate so head-pair 0 finishes last; the final
                    # out-blocks then order their contraction to put
                    # ctx(3,0) last, hiding the normalization chain
                    hp_order = (1, 2, 3, 0)
                per_unit = (len(fill) + 3) // 4 if fill else 0
                for ui, hp in enumerate(hp_order):
                    attention_unit(j, hp)
                    if ui == 0:
                        for ld in loads:
                            load_xt(*ld)
                    for _ in range(per_unit):
                        if fill:
                            f = fill.pop(0)
                            f[0](f[1], f[2])
                    if j == NQT - 1 and ui < 3:
                        # windows 0..2's output projections, deferred to
                        # here: the only independent PE work left to
                        # fill the exp-bound final window
                        for qc in range(4 * ui, 4 * ui + 4):
                            out_block(qc)
            for qc in range(12, 16):
                out_block(qc, on_scalar=True, dvc_order=(1, 2, 3, 0))

    nc.compile()
    return nc


def _in_maps(queries, keys, values, Wq, Wk, Wv, Wo):
    import ml_dtypes

    bf = ml_dtypes.bfloat16
    scale = np.float32(0.125)  # (DK//H) ** -0.5, exact power of two
    xqt = [np.ascontiguousarray(queries[b].T).astype(bf) for b in range(B)]
    xkt = [np.ascontiguousarray(keys[b].T).astype(bf) for b in range(B)]
    xvt = [np.ascontiguousarray(values[b].T).astype(bf) for b in range(B)]
    in_maps = []
    for c in range(N_CORES):
        b, g = divmod(c, 2)
        sl = slice(g * E_LOCAL, (g + 1) * E_LOCAL)
        in_maps.append({
            "xq": xqt[b],
            "xk": xkt[b],
            "xv": xvt[b],
            "wq": np.ascontiguousarray(Wq[:, sl] * scale).astype(bf),
            "wk": np.ascontiguousarray(Wk[:, sl]).astype(bf),
            "wv": np.ascontiguousarray(Wv[:, sl]).astype(bf),
            "wo": np.ascontiguousarray(Wo[sl, :]).astype(bf),
        })
    return in_maps


def kernel(queries, keys, values, mask=None, Wq=None, Wk=None, Wv=None,
           Wo=None, **_ignored):
    from concourse.bass_utils import run_bass_kernel_spmd

    if "nc" not in _cached:
        _cached["nc"] = _build()
    nc = _cached["nc"]

    in_maps = _in_maps(queries, keys, values, Wq, Wk, Wv, Wo)
    res = run_bass_kernel_spmd(nc, in_maps, core_ids=list(range(N_CORES)))
    outs = res.results
    full = np.empty((B, S, HID), np.float32)
    for b in range(B):
        full[b] = (outs[2 * b]["out"].astype(np.float32)
                   + outs[2 * b + 1]["out"].astype(np.float32))
    return full


# revision 32
# speedup vs baseline: 1.1624x; 1.1624x over previous
"""Multi-head causal attention kernel for Trainium2 (8 NeuronCores).

Problem: B=4, S=2048, HID=1024, H=16 heads (head_dim 64), causal mask,
fp32 I/O.  out = softmax(mask + (XqWq)(XkWk)^T/8) (XvWv) Wo

Sharding: 8 cores = 4 batches x 2 head-groups.  Core c handles batch
c//2 and heads (c%2)*8 .. +8 (dk slice of 512).  Each core computes a
full-shape [S, HID] partial output (its head-group's contribution
through Wo); the host sums the two partials per batch.

Host-side prep: X tensors are transposed to [HID, S] and cast to bf16
(with Wq pre-scaled by 1/8) so the kernel needs no PE transposes and
half the HBM traffic.  All matmul operands are bf16 (1 cycle/column on
the PE vs ~2 for fp32); accumulation stays fp32 in PSUM, mask add and
softmax normalization stay fp32, output is fp32.

Per-core dataflow:
  - Project in s-tiles of 512: qT (per-window, [e,s]), kT (persistent
    [e,s], 2 heads per 128-partition tile), v (persistent [s,e] with a
    ones column per head so the PV matmul also emits softmax
    denominators).
  - Attention in transposed [k,q] orientation per (q-window j, head
    pair hp): logits^T = kT-chunk (stationary) x qT (moving) with
    causally-restricted columns; additive triangular mask on diagonal
    blocks (DVE, in PSUM); exp on ScalarE PSUM->SBUF (bf16 out); PV
    accumulates ctx^T in PSUM.  Denominator rows -> one batched
    reciprocal (DVE) -> partition_broadcast (GpSimd) ->
    multiply-evacuate ctx^T to bf16 (DVE).
  - Output projection ctx^T.T @ Wo per q-window; PSUM evacuation on
    DVE (ScalarE stays dedicated to exp); bf16 DMA out, fp32 host sum.

Emission order = Tile scheduling priority.  Projection pieces for
s-tile j+1 interleave with the attention units of window j, so the PE
always has dense independent matmul work while the exp-bound attention
chain waits on ScalarE (keeping the PE HAM activity monitor at the
full 2.4 GHz clock).  All out-projections are deferred to window 3 —
the only window with no projection work left — and the final window
rotates its head-pair order so the last unit's softmax-normalization
chain hides under out-block matmuls whose contraction order puts the
late head-pair last.
"""

import numpy as np

B, S, HID = 4, 2048, 1024
H_LOCAL, E_LOCAL = 8, 512  # heads / dk columns handled per core
N_CORES = 8

_cached = {}


def _build():
    from concourse import bacc
    import concourse.bass as bass
    import concourse.mybir as mybir
    import concourse.tile as tile

    F32 = mybir.dt.float32
    BF16 = mybir.dt.bfloat16
    Exp = mybir.ActivationFunctionType.Exp

    nc = bacc.Bacc()
    # pre-transposed [HID, S] bf16 inputs
    xq = nc.dram_tensor("xq", [HID, S], BF16, kind="ExternalInput")
    xk = nc.dram_tensor("xk", [HID, S], BF16, kind="ExternalInput")
    xv = nc.dram_tensor("xv", [HID, S], BF16, kind="ExternalInput")
    wq = nc.dram_tensor("wq", [HID, E_LOCAL], BF16, kind="ExternalInput")
    wk = nc.dram_tensor("wk", [HID, E_LOCAL], BF16, kind="ExternalInput")
    wv = nc.dram_tensor("wv", [HID, E_LOCAL], BF16, kind="ExternalInput")
    wo = nc.dram_tensor("wo", [E_LOCAL, HID], BF16, kind="ExternalInput")
    # bf16 output: halves the store traffic; the host sums the two
    # per-batch partials in fp32 (quantization ~0.2% of partial
    # magnitude, far under the error budget)
    out = nc.dram_tensor("out", [S, HID], BF16, kind="ExternalOutput")

    NDC = HID // 128       # 8 d-chunks (contraction)
    NEC = E_LOCAL // 128   # 4 e-chunks = head pairs
    NKC = S // 128         # 16 k-chunks
    NQT = 4                # q windows of 512 = s-tiles
    STW = S // NQT         # 512

    with tile.TileContext(nc) as tc:
        with (
            tc.sbuf_pool(name="consts", bufs=1) as consts,
            tc.sbuf_pool(name="persist", bufs=1) as persist,
            tc.sbuf_pool(name="stream", bufs=1) as sm,
            tc.psum_pool(name="ps", bufs=1) as ps,
        ):
            # additive causal mask for diagonal [k,q] blocks: 0 where
            # k <= q else -1e9
            trimask = consts.tile([128, 128], F32)
            nc.gpsimd.memset(trimask, 0.0)
            nc.gpsimd.affine_select(
                out=trimask, in_=trimask,
                compare_op=mybir.AluOpType.is_ge, fill=-1e9, base=0,
                pattern=[[1, 128]], channel_multiplier=-1,
            )
            ones_col = consts.tile([128, 1], BF16)
            nc.vector.memset(ones_col, 1.0)
            # warm the ACT exp table during the initial DMA wait
            warmup = consts.tile([1, 16], F32)
            nc.vector.memset(warmup, 0.0)
            nc.scalar.activation(warmup, warmup, Exp)

            kt_sb = [persist.tile([128, S], BF16, name=f"kt{i}",
                                  tag=f"kt{i}") for i in range(NEC)]
            v_sb = [persist.tile([128, H_LOCAL, 65], BF16, name=f"v{i}",
                                 tag=f"v{i}") for i in range(NKC)]

            wq_sb = sm.tile([128, NDC, E_LOCAL], BF16, tag="wq", bufs=1)
            wk_sb = sm.tile([128, NDC, E_LOCAL], BF16, tag="wk", bufs=1)
            wv_sb = sm.tile([128, NDC, E_LOCAL], BF16, tag="wv", bufs=1)
            wo_sb = sm.tile([128, NEC, HID], BF16, tag="wo", bufs=1)

            qt_rot = {}   # (window, ec) -> [128, 512] bf16 tile
            ctx_rot = {}  # (window, hp) -> [128, 512] bf16 tile
            xt_tiles = {}  # (tensor, st) -> [128, NDC, 512] tile

            def load_w(which):
                src = {"q": (wq, wq_sb), "k": (wk, wk_sb),
                       "v": (wv, wv_sb)}.get(which)
                if src is not None:
                    nc.sync.dma_start(
                        out=src[1],
                        in_=src[0].rearrange("(dc p) e -> p dc e", p=128))
                else:
                    nc.sync.dma_start(
                        out=wo_sb,
                        in_=wo.rearrange("(dv p) n -> p dv n", p=128))

            def load_xt(tname, st):
                xdram = {"q": xq, "k": xk, "v": xv}[tname]
                t = sm.tile([128, NDC, STW], BF16, tag=f"xt{tname}",
                            bufs=2, name=f"xt_{tname}{st}")
                nc.sync.dma_start(
                    out=t,
                    in_=xdram[:, st * STW:(st + 1) * STW].rearrange(
                        "(dc p) s -> p dc s", p=128))
                xt_tiles[(tname, st)] = t

            def proj_q(st, ec):
                xt = xt_tiles[("q", st)]
                pj = ps.tile([128, STW], F32, tag="work", bufs=2,
                             name=f"pjq_{st}_{ec}")
                for dc in range(NDC):
                    nc.tensor.matmul(
                        pj, wq_sb[:, dc, ec * 128:(ec + 1) * 128],
                        xt[:, dc, :],
                        start=(dc == 0), stop=(dc == NDC - 1))
                qt_rot[(st, ec)] = sm.tile([128, STW], BF16,
                                           tag=f"qtr{ec}", bufs=4,
                                           name=f"qtr{ec}_{st}")
                nc.vector.tensor_copy(qt_rot[(st, ec)], pj)

            def proj_k(st, ec):
                xt = xt_tiles[("k", st)]
                pj = ps.tile([128, STW], F32, tag="work", bufs=2,
                             name=f"pjk_{st}_{ec}")
                for dc in range(NDC):
                    nc.tensor.matmul(
                        pj, wk_sb[:, dc, ec * 128:(ec + 1) * 128],
                        xt[:, dc, :],
                        start=(dc == 0), stop=(dc == NDC - 1))
                nc.vector.tensor_copy(
                    kt_sb[ec][:, st * STW:(st + 1) * STW], pj)

            def proj_v(st, sc):
                xt = xt_tiles[("v", st)]
                pv = ps.tile([128, E_LOCAL], F32, tag="work", bufs=2,
                             name=f"pv_{st}_{sc}")
                for dc in range(NDC):
                    nc.tensor.matmul(
                        pv, xt[:, dc, sc * 128:(sc + 1) * 128],
                        wv_sb[:, dc, :],
                        start=(dc == 0), stop=(dc == NDC - 1))
                ci = st * 4 + sc
                nc.vector.tensor_copy(
                    v_sb[ci][:, :, 0:64],
                    pv.rearrange("p (h e) -> p h e", h=H_LOCAL))
                ones_b = bass.AP(
                    tensor=ones_col.tensor, offset=ones_col.offset,
                    ap=[ones_col.ap[0], [0, H_LOCAL], ones_col.ap[1]],
                )
                nc.vector.tensor_copy(v_sb[ci][:, :, 64:65], ones_b)

            def attention_unit(j, hp):
                q0 = j * 512
                nlast = 4 * j + 3
                qt = qt_rot[(j, hp)]
                cpx = [ps.tile([65, 512], F32, tag="cpx", bufs=2,
                               name=f"cpx{hp}_{j}_{hi}")
                       for hi in range(2)]
                ctx_rot[(j, hp)] = sm.tile([128, 512], BF16,
                                           tag=f"ctxr{hp}", bufs=4,
                                           name=f"ctxr{hp}_{j}")
                for c in range(4 * j + 4):
                    vo = max(0, c * 128 - q0)
                    lg = ps.tile([128, 1024], F32, tag="lg", bufs=2,
                                 name=f"lg{hp}_{j}_{c}")
                    pt = sm.tile([128, 1024], BF16, tag="pt", bufs=3,
                                 name=f"pt{hp}_{j}_{c}")
                    for hi in range(2):
                        nc.tensor.matmul(
                            lg[:, hi * 512 + vo:(hi + 1) * 512],
                            kt_sb[hp][hi * 64:(hi + 1) * 64,
                                      c * 128:(c + 1) * 128],
                            qt[hi * 64:(hi + 1) * 64, vo:512],
                            start=True, stop=True)
                    if c >= 4 * j:
                        m = c - 4 * j
                        blk = lg.rearrange("p (hh q) -> p hh q", hh=2)[
                            :, :, m * 128:(m + 1) * 128]
                        tri_b = bass.AP(
                            tensor=trimask.tensor, offset=trimask.offset,
                            ap=[trimask.ap[0], [0, 2], trimask.ap[1]],
                        )
                        nc.vector.tensor_add(blk, blk, tri_b)
                    if vo == 0:
                        nc.scalar.activation(pt, lg, Exp)
                    else:
                        for hi in range(2):
                            nc.scalar.activation(
                                pt[:, hi * 512 + vo:(hi + 1) * 512],
                                lg[:, hi * 512 + vo:(hi + 1) * 512], Exp)
                    for hi in range(2):
                        nc.tensor.matmul(
                            cpx[hi][:, vo:512],
                            v_sb[c][:, hp * 2 + hi, :],
                            pt[:, hi * 512 + vo:(hi + 1) * 512],
                            start=(c == 0), stop=(c == nlast))
                den = sm.tile([1, 1024], F32, tag="den", bufs=2,
                              name=f"den{hp}_{j}")
                for hi in range(2):
                    nc.vector.tensor_copy(
                        den[0:1, hi * 512:(hi + 1) * 512],
                        cpx[hi][64:65, :])
                nc.vector.reciprocal_approx_fast(out=den, in_=den)
                for hi in range(2):
                    bcast = sm.tile([64, 512], F32, tag=f"bcast{hi}",
                                    bufs=2, name=f"bc{hp}_{j}_{hi}")
                    nc.gpsimd.partition_broadcast(
                        bcast, den[0:1, hi * 512:(hi + 1) * 512])
                    nc.vector.tensor_mul(
                        ctx_rot[(j, hp)][hi * 64:(hi + 1) * 64, :],
                        cpx[hi][0:64, :], bcast)

            def out_block(qc, on_scalar=False, dvc_order=(0, 1, 2, 3)):
                for nh in range(2):
                    po = ps.tile([128, 512], F32, tag="work", bufs=2,
                                 name=f"po{qc}_{nh}")
                    for i, dvc in enumerate(dvc_order):
                        nc.tensor.matmul(
                            po,
                            ctx_rot[(qc // 4, dvc)][:,
                                                    (qc % 4) * 128:
                                                    (qc % 4 + 1) * 128],
                            wo_sb[:, dvc, nh * 512:(nh + 1) * 512],
                            start=(i == 0), stop=(i == NEC - 1))
                    osb = sm.tile([128, 512], BF16, tag="osb", bufs=4,
                                  name=f"osb{qc}_{nh}")
                    if on_scalar:
                        # tail blocks: exp is done, ScalarE is idle
                        nc.scalar.copy(osb, po)
                    else:
                        nc.vector.tensor_copy(osb, po)
                    nc.sync.dma_start(
                        out=out[qc * 128:(qc + 1) * 128,
                                nh * 512:(nh + 1) * 512],
                        in_=osb)

            # ---- emission (= scheduling priority) order ----
            # Prologue: weights + s-tile 0, with attention(0,0)'s
            # dependencies (q0/k0 head-pair 0, all v) first.  Weight
            # loads interleave with x-tile loads so the first
            # projection can start as early as possible.
            # wq + x_q tile 0 load in halves so the first projection's
            # dc 0..3 matmuls start after ~1MB of DMA instead of ~2MB
            xtq0 = sm.tile([128, NDC, STW], BF16, tag="xtq", bufs=2,
                           name="xt_q0")
            xt_tiles[("q", 0)] = xtq0
            for h in range(2):
                dcs = slice(h * 4, h * 4 + 4)
                rows = slice(h * 512, h * 512 + 512)
                nc.sync.dma_start(
                    out=wq_sb[:, dcs, :],
                    in_=wq[rows, :].rearrange("(dc p) e -> p dc e", p=128))
                nc.sync.dma_start(
                    out=xtq0[:, dcs, :],
                    in_=xq[rows, 0:STW].rearrange(
                        "(dc p) s -> p dc s", p=128))
            load_w("k")
            load_xt("k", 0)
            load_w("v")
            load_xt("v", 0)
            load_w("o")
            # all q/k projection pieces for s-tile 0 before the first
            # attention unit: the PE stream is in-order, so anything
            # behind attention(0,0) stalls on the x_v DMA otherwise
            for ec in range(NEC):
                proj_q(0, ec)
                proj_k(0, ec)
            for sc in range(4):
                proj_v(0, sc)

            for j in range(NQT):
                if j < NQT - 1:
                    st = j + 1
                    loads = [("q", st), ("k", st), ("v", st)]
                    fill = [
                        (proj_q, st, 0), (proj_k, st, 0),
                        (proj_v, st, 0), (proj_v, st, 1),
                        (proj_v, st, 2), (proj_v, st, 3),
                        (proj_q, st, 1), (proj_k, st, 1),
                        (proj_q, st, 2), (proj_k, st, 2),
                        (proj_q, st, 3), (proj_k, st, 3),
                    ]
                    hp_order = range(NEC)
                else:
                    loads = []
                    fill = []
                    # rotate so head-pair 0 finishes last; the final
                    # out-blocks then order their contraction to put
                    # ctx(3,0) last, hiding the normalization chain
                    hp_order = (1, 2, 3, 0)
                per_unit = (len(fill) + 3) // 4 if fill else 0
                for ui, hp in enumerate(hp_order):
                    attention_unit(j, hp)
                    if ui == 0:
                        for ld in loads:
                            load_xt(*ld)
                    for _ in range(per_unit):
                        if fill:
                            f = fill.pop(0)
                            f[0](f[1], f[2])
                    if j == NQT - 1 and ui < 3:
                        # windows 0..2's output projections, deferred to
                        # here: the only independent PE work left to
                        # fill the exp-bound final window
                        for qc in range(4 * ui, 4 * ui + 4):
                            out_block(qc)
            # Final four out-blocks split their contraction: head-pairs
            # 1..3 accumulate and evacuate via ScalarE (exp is done)
            # while the last unit's normalization chain still runs;
            # only head-pair 0's single matmul + a DVE add follow it.
            partials = {}
            for qc in range(12, 16):
                for nh in range(2):
                    po = ps.tile([128, 512], F32, tag="work", bufs=2,
                                 name=f"poA{qc}_{nh}")
                    for i, dvc in enumerate((1, 2, 3)):
                        nc.tensor.matmul(
                            po,
                            ctx_rot[(3, dvc)][:, (qc % 4) * 128:
                                              (qc % 4 + 1) * 128],
                            wo_sb[:, dvc, nh * 512:(nh + 1) * 512],
                            start=(i == 0), stop=(i == 2))
                    opart = sm.tile([128, 512], F32, tag="opart",
                                    bufs=8, name=f"opart{qc}_{nh}")
                    nc.scalar.copy(opart, po)
                    partials[(qc, nh)] = opart
            for qc in range(12, 16):
                for nh in range(2):
                    po2 = ps.tile([128, 512], F32, tag="work", bufs=2,
                                  name=f"poB{qc}_{nh}")
                    nc.tensor.matmul(
                        po2,
                        ctx_rot[(3, 0)][:, (qc % 4) * 128:
                                        (qc % 4 + 1) * 128],
                        wo_sb[:, 0, nh * 512:(nh + 1) * 512],
                        start=True, stop=True)
                    osb = sm.tile([128, 512], BF16, tag="osb", bufs=4,
                                  name=f"osb{qc}_{nh}")
                    nc.vector.tensor_add(osb, po2, partials[(qc, nh)])
                    nc.sync.dma_start(
                        out=out[qc * 128:(qc + 1) * 128,
                                nh * 512:(nh + 1) * 512],
                        in_=osb)

    nc.compile()
    return nc


def _in_maps(queries, keys, values, Wq, Wk, Wv, Wo):
    import ml_dtypes

    bf = ml_dtypes.bfloat16
    scale = np.float32(0.125)  # (DK//H) ** -0.5, exact power of two
    xqt = [np.ascontiguousarray(queries[b].T).astype(bf) for b in range(B)]
    xkt = [np.ascontiguousarray(keys[b].T).astype(bf) for b in range(B)]
    xvt = [np.ascontiguousarray(values[b].T).astype(bf) for b in range(B)]
    in_maps = []
    for c in range(N_CORES):
        b, g = divmod(c, 2)
        sl = slice(g * E_LOCAL, (g + 1) * E_LOCAL)
        in_maps.append({
            "xq": xqt[b],
            "xk": xkt[b],
            "xv": xvt[b],
            "wq": np.ascontiguousarray(Wq[:, sl] * scale).astype(bf),
            "wk": np.ascontiguousarray(Wk[:, sl]).astype(bf),
            "wv": np.ascontiguousarray(Wv[:, sl]).astype(bf),
            "wo": np.ascontiguousarray(Wo[sl, :]).astype(bf),
        })
    return in_maps


def kernel(queries, keys, values, mask=None, Wq=None, Wk=None, Wv=None,
           Wo=None, **_ignored):
    from concourse.bass_utils import run_bass_kernel_spmd

    if "nc" not in _cached:
        _cached["nc"] = _build()
    nc = _cached["nc"]

    in_maps = _in_maps(queries, keys, values, Wq, Wk, Wv, Wo)
    res = run_bass_kernel_spmd(nc, in_maps, core_ids=list(range(N_CORES)))
    outs = res.results
    full = np.empty((B, S, HID), np.float32)
    for b in range(B):
        full[b] = (outs[2 * b]["out"].astype(np.float32)
                   + outs[2 * b + 1]["out"].astype(np.float32))
    return full
